# revision 6
# baseline (speedup 1.0000x reference)
"""AttnBlock (GroupNorm -> q/k/v 1x1 conv -> full spatial attention -> out proj
-> residual) for Trainium2, sharded over 8 NeuronCores.

Sharding: 8 cores = 4 batches x 2 query-halves. Each core gets its batch's
full x (columns rotated so its 2048 query positions come first), computes
GroupNorm + k/v over all 4096 positions and attention for its 2048 queries.

v2 design, all big matmuls in fp8e4 DoubleRow (0.5 cycles/row -- 2x the
fp32r/bf16 PE rate, 256-deep contraction per instruction):
  - x is loaded from HBM exactly once and stays SBUF-resident (8MB): feeds
    stats, the projections and the residual add. No q DRAM scratch.
  - wq/wk/wv are host-prescaled by 32 so their fp8 encodings stay in the
    normal range (raw entries ~N(0, 1/512) would land subnormal); the 1/32
    is folded into the PSUM->fp8 cast of q/k/v. wo is cast raw.
  - q8/k8 keep channels paired for DoubleRow ([128, cc, :] layout), vT8 is
    token-major so PV pairs adjacent key chunks.
  - attention per 512-query group: per key-chunk pair, two logits DR
    matmuls + ACT exp (scale s/1024 folds the weight prescale; bias -ln16
    keeps exp outputs below fp8e4's 240 max) writing the two fp8 P tiles of
    an expair; then one ones-lhsT DR matmul accumulates the softmax
    denominator in PSUM and four DR matmuls accumulate P.V -- no DVE work
    in the steady state.
  - at each group boundary pv PSUM drains to bf16 immediately (frees the 4
    banks for the next group); the normalize (broadcast reciprocal of the
    denominator) and the fp8 out-projection tail are spliced into the next
    group's instruction stream so the PE never waits on ACT/DVE latency.
"""

import numpy as np
import ml_dtypes

import bass_rust
import concourse.bass as bass
import concourse.tile as tile
from concourse import mybir
from concourse.bass_utils import run_bass_kernel_spmd

B, C, H, W = 4, 512, 64, 64
HW = H * W            # 4096
HALF = HW // 2        # 2048 query positions per core
NG = 32               # groups
GS = C // NG          # 16 channels per group
EPS = 1e-6
P = 128               # SBUF partitions
NCC = C // P          # 4 channel chunks
JT = 512              # projection j-tile width
NJT = HW // JT        # 8
NJC = HW // P         # 32 j-chunks of 128
IGW = 512             # query-group width
NIG = HALF // IGW     # 4
SCALE = 1.0 / float(np.sqrt(C))
WS = 32.0             # host-side prescale on wq/wk/wv
EXP_SCALE = SCALE  # q8/k8 are cast back to true scale (the /WS is in the cast)
EXP_BIAS = -float(np.log(16.0))  # exp outputs stay < 240 (fp8e4 max)
F32 = mybir.dt.float32
F8 = mybir.dt.float8e4
BF16 = mybir.dt.bfloat16
R = mybir.dt.float32r

AF = mybir.ActivationFunctionType
ALU = mybir.AluOpType
DR = mybir.MatmulPerfMode.DoubleRow

E4NP = ml_dtypes.float8_e4m3  # TRN fp8e4: max normal 240


def _split_drain_waits(nc, max_waits=1):
    """walrus on this container rejects ANY instruction carrying more than one
    sem wait; spill the excess onto same-engine NoOps inserted just before
    (the engine executes the NoOp's waits, then the instruction's remaining
    one -- identical semantics)."""
    uid = [0]
    nsplit = 0
    for f in nc.m.functions:
        for bb in f.blocks:
            insts = bb.instructions
            i = 0
            while i < len(insts):
                inst = insts[i]
                si = getattr(inst, "sync_info", None)
                if si is not None and si.on_wait and len(si.on_wait) > max_waits:
                    waits = list(si.on_wait)
                    keep, rest = waits[-max_waits:], waits[:-max_waits]
                    new_insts = []
                    for j in range(0, len(rest), max_waits):
                        nop = mybir.InstNoOp(
                            name=f"wait-split-{uid[0]}", ins=[], outs=[]
                        )
                        uid[0] += 1
                        nop.engine = inst.engine
                        nop.sync_info = bass_rust.SyncInfo(
                            on_wait=rest[j : j + max_waits], on_update=[]
                        )
                        new_insts.append(nop)
                    inst.sync_info = bass_rust.SyncInfo(
                        on_wait=keep, on_update=list(si.on_update)
                    )
                    for k, nop in enumerate(new_insts):
                        insts.insert(i + k, nop)
                    i += len(new_insts)
                    nsplit += 1
                i += 1
    return nsplit


def build():
    nc = bass.Bass()
    xb = nc.dram_tensor("xb", [C, HW], F32, kind="ExternalInput")
    wq8d = nc.dram_tensor("wq8d", [P, NCC, C], F8, kind="ExternalInput")
    wk8d = nc.dram_tensor("wk8d", [P, NCC, C], F8, kind="ExternalInput")
    wv8d = nc.dram_tensor("wv8d", [P, NCC, C], F8, kind="ExternalInput")
    wo8d = nc.dram_tensor("wo8d", [P, NCC, C], F8, kind="ExternalInput")
    gnw = nc.dram_tensor("gnw", [C], F32, kind="ExternalInput")
    gnb = nc.dram_tensor("gnb", [C], F32, kind="ExternalInput")
    membd = nc.dram_tensor("membd", [P, NCC, NG], F32, kind="ExternalInput")
    bcd = nc.dram_tensor("bcd", [NG, NCC, P], F32, kind="ExternalInput")
    outd = nc.dram_tensor("out", [C, HALF], F32, kind="ExternalOutput")

    with tile.TileContext(nc) as tc, nc.allow_low_precision(
        reason="fp8e4 DoubleRow matmuls validated offline at rel err ~4e-3"
    ):
        with tc.tile_pool(name="pers", bufs=1) as pers:
            # ---- persistent tiles ----
            x_sb = pers.tile([P, NCC, HW], F32, tag="x")      # 8MB, resident
            k8 = pers.tile([P, NCC, HW], F8, tag="k8")
            q8 = pers.tile([P, NCC, HALF], F8, tag="q8")
            vT8 = pers.tile([P, NJC, JT], F8, tag="vT8")
            wq8 = pers.tile([P, NCC, C], F8, tag="wq8")
            wk8 = pers.tile([P, NCC, C], F8, tag="wk8")
            wv8 = pers.tile([P, NCC, C], F8, tag="wv8")
            wo8 = pers.tile([P, NCC, C], F8, tag="wo8")
            gnw_t = pers.tile([P, NCC], F32, tag="gnw")
            gnb_t = pers.tile([P, NCC], F32, tag="gnb")
            gscale = pers.tile([P, NCC], F32, tag="gsc")
            gshift = pers.tile([P, NCC], F32, tag="gsh")
            memb = pers.tile([P, NCC, NG], F32, tag="memb")
            bcm = pers.tile([NG, NCC, P], F32, tag="bc")
            # fp8 ones for the denominator matmul: dual-fp8 LDWEIGHTS rejects
            # single-column weights, so use a full [P, 2, 128] block -- the
            # denominator then lands PSUM-broadcast across all 128 partitions,
            # which also removes the separate broadcast matmul. (Memset can't
            # write fp8 directly; cast from an f32 memset.)
            ones8 = pers.tile([P, 2, P], F8, tag="ones8")
            onesf = pers.tile([P, 2 * P], F32, tag="onesf")
            ebias = pers.tile([P, 1], F32, tag="ebias")
            nc.vector.memset(onesf, 1.0)
            nc.vector.tensor_copy(
                out=ones8,
                in_=onesf[:, 0 : 2 * P].rearrange("p (two m) -> p two m", two=2),
            )
            nc.vector.memset(ebias, EXP_BIAS)

            # ---- phase A: x load (once, 16 tiles) THEN weights; stats on
            # the fly. DMA issues go on the sync+tensor queues (both idle
            # here) so GpSimd is free to be a third stats worker; the two
            # 8MB passes (sum, sum-of-squares) split ACT/DVE/GpSimd so the
            # head tracks the DMA arrival instead of one engine's serial
            # throughput. ----
            XT = 1024
            for cc in range(NCC):
                for jt in range(4):
                    eng = nc.sync if (cc * 4 + jt) % 2 == 0 else nc.scalar
                    eng.dma_start(
                        out=x_sb[:, cc, jt * XT : (jt + 1) * XT],
                        in_=xb.ap()[cc * P : (cc + 1) * P, jt * XT : (jt + 1) * XT],
                    )
            nc.sync.dma_start(out=gnw_t, in_=gnw.ap().rearrange("(a p) -> p a", p=P))
            nc.sync.dma_start(out=gnb_t, in_=gnb.ap().rearrange("(a p) -> p a", p=P))
            nc.scalar.dma_start(out=memb, in_=membd.ap())
            nc.scalar.dma_start(out=bcm, in_=bcd.ap())
            nc.sync.dma_start(out=wq8, in_=wq8d.ap())
            nc.scalar.dma_start(out=wk8, in_=wk8d.ap())
            nc.sync.dma_start(out=wv8, in_=wv8d.ap())
            nc.scalar.dma_start(out=wo8, in_=wo8d.ap())

            with (
                tc.tile_pool(name="statq", bufs=3) as sq_pool,
                tc.tile_pool(name="statsm", bufs=1) as sm,
                tc.tile_pool(name="statps", bufs=1, space="PSUM") as sps,
            ):
                gps = sps.tile([NG, 2], F32, tag="gstat")
                for cc in range(NCC):
                    s1t = sm.tile([P, 4], F32, tag=f"s1{cc}", name=f"s1{cc}")
                    s2t = sm.tile([P, 4], F32, tag=f"s2{cc}", name=f"s2{cc}")
                    for jt in range(4):
                        xsl = x_sb[:, cc, jt * XT : (jt + 1) * XT]
                        nc.vector.reduce_sum(
                            out=s1t[:, jt : jt + 1], in_=xsl, axis=mybir.AxisListType.X
                        )
                        sqw = sq_pool.tile([P, XT], F32, tag="sqw", name="sqw")
                        nc.scalar.activation(
                            out=sqw,
                            in_=xsl,
                            func=AF.Square,
                            accum_out=s2t[:, jt : jt + 1],
                        )
                    mm2 = sm.tile([P, 2], F32, tag=f"m2{cc}", name=f"m2{cc}")
                    m1r = sm.tile([P, 1], F32, tag=f"m1r{cc}", name=f"m1r{cc}")
                    nc.vector.reduce_sum(out=m1r, in_=s1t, axis=mybir.AxisListType.X)
                    nc.vector.tensor_scalar_mul(mm2[:, 0:1], m1r, 1.0 / HW)
                    m2r = sm.tile([P, 1], F32, tag=f"m2r{cc}", name=f"m2r{cc}")
                    nc.vector.reduce_sum(out=m2r, in_=s2t, axis=mybir.AxisListType.X)
                    nc.vector.tensor_scalar_mul(mm2[:, 1:2], m2r, 1.0 / HW)
                    nc.tensor.matmul(
                        gps,
                        memb[:, cc, :],
                        mm2,
                        start=(cc == 0),
                        stop=(cc == NCC - 1),
                    )
                # group stats -> per-channel scale/shift
                gs = sm.tile([NG, 2], F32, tag="gs")
                nc.scalar.mul(gs, gps, 1.0 / GS)
                sqg = sm.tile([NG, 1], F32, tag="sqg")
                nc.vector.tensor_mul(sqg, gs[:, 0:1], gs[:, 0:1])
                varg = sm.tile([NG, 1], F32, tag="varg")
                nc.vector.tensor_sub(varg, gs[:, 1:2], sqg)
                g2 = sm.tile([NG, 2], F32, tag="g2")
                nc.vector.tensor_copy(g2[:, 0:1], gs[:, 0:1])
                sbeps = sm.tile([NG, 1], F32, tag="eps")
                nc.vector.memset(sbeps, EPS)
                nc.scalar.activation(
                    out=g2[:, 1:2], in_=varg, func=AF.Sqrt, bias=sbeps
                )
                nc.vector.reciprocal(out=g2[:, 1:2], in_=g2[:, 1:2])
                for cc in range(NCC):
                    chp = sps.tile([P, 2], F32, tag="chs", name="chs")
                    nc.tensor.matmul(chp, bcm[:, cc, :], g2, start=True, stop=True)
                    nc.vector.tensor_mul(
                        gscale[:, cc : cc + 1], chp[:, 1:2], gnw_t[:, cc : cc + 1]
                    )
                    tmpm = sm.tile([P, 1], F32, tag="tm", name="tm")
                    nc.vector.tensor_mul(tmpm, chp[:, 0:1], gscale[:, cc : cc + 1])
                    nc.vector.tensor_sub(
                        gshift[:, cc : cc + 1], gnb_t[:, cc : cc + 1], tmpm
                    )

            # ---- phase C: projections (k, vT, q), all fp8 DoubleRow ----
            with (
                tc.tile_pool(name="projx", bufs=2) as px,
                tc.tile_pool(name="projps", bufs=3, space="PSUM") as pps,
            ):
                for jt in range(NJT):
                    jsl = slice(jt * JT, (jt + 1) * JT)
                    xn8 = px.tile([P, NCC, JT], F8, tag="xn", name="xn")
                    for cc in range(NCC):
                        # SBUF->SBUF, so GpSimd can own it (PSUM is off-limits
                        # to Pool); frees DVE/ACT for the PSUM->fp8 casts
                        nc.gpsimd.tensor_scalar(
                            out=xn8[:, cc, :],
                            in0=x_sb[:, cc, jsl],
                            scalar1=gscale[:, cc : cc + 1],
                            scalar2=gshift[:, cc : cc + 1],
                            op0=ALU.mult,
                            op1=ALU.add,
                        )
                    # k tiles (feature-major)
                    for oc in range(NCC):
                        kps = pps.tile([P, JT], F32, tag="kq", name="kps")
                        for c2 in range(2):
                            nc.tensor.matmul(
                                kps,
                                wk8[:, 2 * c2 : 2 * c2 + 2, oc * P : (oc + 1) * P],
                                xn8[:, 2 * c2 : 2 * c2 + 2, :],
                                start=(c2 == 0),
                                stop=(c2 == 1),
                                perf_mode=DR,
                            )
                        nc.scalar.mul(k8[:, oc, jsl], kps, 1.0 / WS)
                    # vT tiles (token-major)
                    for js in range(4):
                        vps = pps.tile([P, C], F32, tag="v", name="vps")
                        for c2 in range(2):
                            nc.tensor.matmul(
                                vps,
                                xn8[:, 2 * c2 : 2 * c2 + 2, js * P : (js + 1) * P],
                                wv8[:, 2 * c2 : 2 * c2 + 2, :],
                                start=(c2 == 0),
                                stop=(c2 == 1),
                                perf_mode=DR,
                            )
                        nc.vector.tensor_scalar_mul(
                            vT8[:, jt * 4 + js, :], vps, 1.0 / WS
                        )
                    # q tiles (first half only = our queries)
                    if jt < NJT // 2:
                        for oc in range(NCC):
                            qps = pps.tile([P, JT], F32, tag="kq", name="qps")
                            for c2 in range(2):
                                nc.tensor.matmul(
                                    qps,
                                    wq8[:, 2 * c2 : 2 * c2 + 2, oc * P : (oc + 1) * P],
                                    xn8[:, 2 * c2 : 2 * c2 + 2, :],
                                    start=(c2 == 0),
                                    stop=(c2 == 1),
                                    perf_mode=DR,
                                )
                            if oc % 2 == 0:
                                nc.scalar.mul(q8[:, oc, jsl], qps, 1.0 / WS)
                            else:
                                nc.vector.tensor_scalar_mul(
                                    q8[:, oc, jsl], qps, 1.0 / WS
                                )

            # ---- phase D: attention + out projection + residual ----
            with (
                tc.tile_pool(name="attne", bufs=3) as ae,
                tc.tile_pool(name="attnsb", bufs=2) as asb,
                tc.tile_pool(name="attnps", bufs=3, space="PSUM") as aps,
                tc.tile_pool(name="pvps", bufs=1, space="PSUM") as pvp_pool,
                tc.tile_pool(name="dnps", bufs=1, space="PSUM") as dnp_pool,
            ):
                pending_norm = None
                pending_tail = None
                for ig in range(NIG):
                    isl = slice(ig * IGW, (ig + 1) * IGW)
                    pvp = [
                        pvp_pool.tile([P, IGW], F32, tag=f"pv{cc}", name=f"pv{cc}")
                        for cc in range(NCC)
                    ]
                    dnp = dnp_pool.tile([P, IGW], F32, tag="dn")
                    for jc2 in range(NJC // 2):
                        expair = ae.tile([P, 2, IGW], F8, tag="ex", name="expair")
                        for hf in range(2):
                            jc = 2 * jc2 + hf
                            ap_t = aps.tile([P, IGW], F32, tag="attn", name="ap_t")
                            for c2 in range(2):
                                nc.tensor.matmul(
                                    ap_t,
                                    k8[:, 2 * c2 : 2 * c2 + 2, jc * P : (jc + 1) * P],
                                    q8[:, 2 * c2 : 2 * c2 + 2, isl],
                                    start=(c2 == 0),
                                    stop=(c2 == 1),
                                    perf_mode=DR,
                                )
                            nc.scalar.activation(
                                out=expair[:, hf, :],
                                in_=ap_t,
                                func=AF.Exp,
                                scale=EXP_SCALE,
                                bias=ebias,
                            )
                        if jc2 == 0 and pending_norm is not None:
                            pending_norm()
                            pending_norm = None
                        if jc2 == 2 and pending_tail is not None:
                            pending_tail()
                            pending_tail = None
                        nc.tensor.matmul(
                            dnp,
                            ones8,
                            expair,
                            start=(jc2 == 0),
                            stop=(jc2 == NJC // 2 - 1),
                            perf_mode=DR,
                        )
                        for cc in range(NCC):
                            nc.tensor.matmul(
                                pvp[cc],
                                vT8[:, 2 * jc2 : 2 * jc2 + 2, cc * P : (cc + 1) * P],
                                expair,
                                start=(jc2 == 0),
                                stop=(jc2 == NJC // 2 - 1),
                                perf_mode=DR,
                            )
                    # ---- ig boundary: drain pv PSUM to bf16 immediately (no
                    # dependency on the slow DVE reciprocal), then normalize
                    # from the staged copy inside the next ig's stream. dnp is
                    # already broadcast across partitions (ones8 lhsT).
                    pvraw = asb.tile([P, NCC, IGW], BF16, tag="pvraw", name="pvraw")
                    for cc in range(NCC):
                        nc.vector.tensor_copy(out=pvraw[:, cc, :], in_=pvp[cc])
                    pvn8 = asb.tile([P, NCC, IGW], F8, tag="pvn", name="pvn8")

                    def make_norm(dnp=dnp, pvraw=pvraw, pvn8=pvn8):
                        def norm():
                            recipb = asb.tile(
                                [P, IGW], F32, tag="recip", name="recipb"
                            )
                            nc.vector.reciprocal(out=recipb, in_=dnp)
                            for cc in range(NCC):
                                nc.vector.tensor_mul(
                                    pvn8[:, cc, :], pvraw[:, cc, :], recipb
                                )
                        return norm

                    def make_tail(isl=isl, pvn8=pvn8):
                        def tail():
                            for oc in range(NCC):
                                oop = aps.tile([P, IGW], F32, tag="attn", name="oop")
                                for c2 in range(2):
                                    nc.tensor.matmul(
                                        oop,
                                        wo8[
                                            :,
                                            2 * c2 : 2 * c2 + 2,
                                            oc * P : (oc + 1) * P,
                                        ],
                                        pvn8[:, 2 * c2 : 2 * c2 + 2, :],
                                        start=(c2 == 0),
                                        stop=(c2 == 1),
                                        perf_mode=DR,
                                    )
                                fo = asb.tile([P, IGW], F32, tag="fo", name="fo")
                                nc.vector.tensor_add(fo, oop, x_sb[:, oc, isl])
                                nc.gpsimd.dma_start(
                                    out=outd.ap()[oc * P : (oc + 1) * P, isl],
                                    in_=fo,
                                )
                        return tail

                    pending_norm = make_norm()
                    pending_tail = make_tail()
                pending_norm()
                pending_tail()

    return nc


_NC_CACHE = {}


def _get_module():
    if "nc" not in _NC_CACHE:
        nc = build()
        _split_drain_waits(nc)  # only needed for walrus codegen, not CoreSim
        _NC_CACHE["nc"] = nc
    return _NC_CACHE["nc"]


def _memb_np():
    m = np.zeros((P, NCC, NG), np.float32)
    for p in range(P):
        for cc in range(NCC):
            m[p, cc, cc * 8 + p // GS] = 1.0
    return m


def _bc_np():
    b = np.zeros((NG, NCC, P), np.float32)
    for cc in range(NCC):
        for p in range(P):
            b[cc * 8 + p // GS, cc, p] = 1.0
    return b


def _w8(w, scale):
    """w [C_out, C_in] f32 -> [P, NCC, C_out] fp8 tile: w8[p, cc, o] =
    (scale * w)[o, cc*128+p]."""
    wT = (np.asarray(w, np.float32) * scale).T  # [C_in, C_out]
    return np.ascontiguousarray(
        wT.reshape(NCC, P, C).transpose(1, 0, 2).astype(E4NP)
    )


def make_in_maps(inputs):
    x = np.asarray(inputs["x"], np.float32).reshape(B, C, HW)
    shared = {
        "wq8d": _w8(inputs["wq"], WS),
        "wk8d": _w8(inputs["wk"], WS),
        "wv8d": _w8(inputs["wv"], WS),
        "wo8d": _w8(inputs["wo"], 1.0),
        "gnw": np.ascontiguousarray(np.asarray(inputs["gn_w"], np.float32)),
        "gnb": np.ascontiguousarray(np.asarray(inputs["gn_b"], np.float32)),
        "membd": _memb_np(),
        "bcd": _bc_np(),
    }
    in_maps = []
    for core in range(8):
        b, h = core // 2, core % 2
        xbm = x[b]
        if h == 1:
            xbm = np.concatenate([xbm[:, HALF:], xbm[:, :HALF]], axis=1)
        in_maps.append({"xb": np.ascontiguousarray(xbm), **shared})
    return in_maps


def assemble(results):
    out = np.empty((B, C, HW), np.float32)
    for core in range(8):
        b, h = core // 2, core % 2
        out[b][:, h * HALF : (h + 1) * HALF] = results[core]["out"]
    return out.reshape(B, C, H, W)


def run_spmd(inputs, trace=False):
    nc = _get_module()
    res = run_bass_kernel_spmd(
        nc, make_in_maps(inputs), core_ids=list(range(8)), trace=trace
    )
    return assemble(res.results), res


def kernel(**inputs) -> np.ndarray:
    out, _ = run_spmd(inputs)
    return out


# revision 7
# speedup vs baseline: 1.0631x; 1.0631x over previous
"""AttnBlock (GroupNorm -> q/k/v 1x1 conv -> full spatial attention -> out proj
-> residual) for Trainium2, sharded over 8 NeuronCores.

Sharding: 8 cores = 4 batches x 2 query-halves. Each core gets its batch's
full x (columns rotated so its 2048 query positions come first), computes
GroupNorm + k/v over all 4096 positions and attention for its 2048 queries.

v2 design, all big matmuls in fp8e4 DoubleRow (0.5 cycles/row -- 2x the
fp32r/bf16 PE rate, 256-deep contraction per instruction):
  - x is loaded from HBM exactly once and stays SBUF-resident (8MB): feeds
    stats, the projections and the residual add. No q DRAM scratch.
  - wq/wk/wv are host-prescaled by 32 so their fp8 encodings stay in the
    normal range (raw entries ~N(0, 1/512) would land subnormal); the 1/32
    is folded into the PSUM->fp8 cast of q/k/v. wo is cast raw.
  - q8/k8 keep channels paired for DoubleRow ([128, cc, :] layout), vT8 is
    token-major so PV pairs adjacent key chunks.
  - attention per 512-query group: per key-chunk pair, two logits DR
    matmuls + ACT exp (scale s/1024 folds the weight prescale; bias -ln16
    keeps exp outputs below fp8e4's 240 max) writing the two fp8 P tiles of
    an expair; then one ones-lhsT DR matmul accumulates the softmax
    denominator in PSUM and four DR matmuls accumulate P.V -- no DVE work
    in the steady state.
  - at each group boundary pv PSUM drains to bf16 immediately (frees the 4
    banks for the next group); the normalize (broadcast reciprocal of the
    denominator) and the fp8 out-projection tail are spliced into the next
    group's instruction stream so the PE never waits on ACT/DVE latency.
"""

import numpy as np
import ml_dtypes

import bass_rust
import concourse.bass as bass
import concourse.tile as tile
from concourse import mybir
from concourse.bass_utils import run_bass_kernel_spmd

B, C, H, W = 4, 512, 64, 64
HW = H * W            # 4096
HALF = HW // 2        # 2048 query positions per core
NG = 32               # groups
GS = C // NG          # 16 channels per group
EPS = 1e-6
P = 128               # SBUF partitions
NCC = C // P          # 4 channel chunks
JT = 512              # projection j-tile width
NJT = HW // JT        # 8
NJC = HW // P         # 32 j-chunks of 128
IGW = 512             # query-group width
NIG = HALF // IGW     # 4
SCALE = 1.0 / float(np.sqrt(C))
WS = 32.0             # host-side prescale on wq/wk/wv
EXP_SCALE = SCALE  # q8/k8 are cast back to true scale (the /WS is in the cast)
EXP_BIAS = -float(np.log(16.0))  # exp outputs stay < 240 (fp8e4 max)
F32 = mybir.dt.float32
F8 = mybir.dt.float8e4
BF16 = mybir.dt.bfloat16
R = mybir.dt.float32r

AF = mybir.ActivationFunctionType
ALU = mybir.AluOpType
DR = mybir.MatmulPerfMode.DoubleRow

E4NP = ml_dtypes.float8_e4m3  # TRN fp8e4: max normal 240


def _split_drain_waits(nc, max_waits=1):
    """walrus on this container rejects ANY instruction carrying more than one
    sem wait; spill the excess onto same-engine NoOps inserted just before
    (the engine executes the NoOp's waits, then the instruction's remaining
    one -- identical semantics)."""
    uid = [0]
    nsplit = 0
    for f in nc.m.functions:
        for bb in f.blocks:
            insts = bb.instructions
            i = 0
            while i < len(insts):
                inst = insts[i]
                si = getattr(inst, "sync_info", None)
                if si is not None and si.on_wait and len(si.on_wait) > max_waits:
                    waits = list(si.on_wait)
                    keep, rest = waits[-max_waits:], waits[:-max_waits]
                    new_insts = []
                    for j in range(0, len(rest), max_waits):
                        nop = mybir.InstNoOp(
                            name=f"wait-split-{uid[0]}", ins=[], outs=[]
                        )
                        uid[0] += 1
                        nop.engine = inst.engine
                        nop.sync_info = bass_rust.SyncInfo(
                            on_wait=rest[j : j + max_waits], on_update=[]
                        )
                        new_insts.append(nop)
                    inst.sync_info = bass_rust.SyncInfo(
                        on_wait=keep, on_update=list(si.on_update)
                    )
                    for k, nop in enumerate(new_insts):
                        insts.insert(i + k, nop)
                    i += len(new_insts)
                    nsplit += 1
                i += 1
    return nsplit


def build():
    nc = bass.Bass()
    xb = nc.dram_tensor("xb", [C, HW], F32, kind="ExternalInput")
    wq8d = nc.dram_tensor("wq8d", [P, NCC, C], F8, kind="ExternalInput")
    wk8d = nc.dram_tensor("wk8d", [P, NCC, C], F8, kind="ExternalInput")
    wv8d = nc.dram_tensor("wv8d", [P, NCC, C], F8, kind="ExternalInput")
    wo8d = nc.dram_tensor("wo8d", [P, NCC, C], F8, kind="ExternalInput")
    gnw = nc.dram_tensor("gnw", [C], F32, kind="ExternalInput")
    gnb = nc.dram_tensor("gnb", [C], F32, kind="ExternalInput")
    membd = nc.dram_tensor("membd", [P, NCC, NG], F32, kind="ExternalInput")
    bcd = nc.dram_tensor("bcd", [NG, NCC, P], F32, kind="ExternalInput")
    outd = nc.dram_tensor("out", [C, HALF], F32, kind="ExternalOutput")

    with tile.TileContext(nc) as tc, nc.allow_low_precision(
        reason="fp8e4 DoubleRow matmuls validated offline at rel err ~4e-3"
    ):
        with tc.tile_pool(name="pers", bufs=1) as pers:
            # ---- persistent tiles ----
            x_sb = pers.tile([P, NCC, HW], F32, tag="x")      # 8MB, resident
            k8 = pers.tile([P, NCC, HW], F8, tag="k8")
            q8 = pers.tile([P, NCC, HALF], F8, tag="q8")
            vT8 = pers.tile([P, NJC, JT], F8, tag="vT8")
            wq8 = pers.tile([P, NCC, C], F8, tag="wq8")
            wk8 = pers.tile([P, NCC, C], F8, tag="wk8")
            wv8 = pers.tile([P, NCC, C], F8, tag="wv8")
            wo8 = pers.tile([P, NCC, C], F8, tag="wo8")
            gnw_t = pers.tile([P, NCC], F32, tag="gnw")
            gnb_t = pers.tile([P, NCC], F32, tag="gnb")
            gscale = pers.tile([P, NCC], F32, tag="gsc")
            gshift = pers.tile([P, NCC], F32, tag="gsh")
            memb = pers.tile([P, NCC, NG], F32, tag="memb")
            bcm = pers.tile([NG, NCC, P], F32, tag="bc")
            # fp8 ones for the denominator matmul: dual-fp8 LDWEIGHTS rejects
            # single-column weights, so use a full [P, 2, 128] block -- the
            # denominator then lands PSUM-broadcast across all 128 partitions,
            # which also removes the separate broadcast matmul. (Memset can't
            # write fp8 directly; cast from an f32 memset.)
            ones8 = pers.tile([P, 2, P], F8, tag="ones8")
            onesf = pers.tile([P, 2 * P], F32, tag="onesf")
            ebias = pers.tile([P, 1], F32, tag="ebias")
            nc.vector.memset(onesf, 1.0)
            nc.vector.tensor_copy(
                out=ones8,
                in_=onesf[:, 0 : 2 * P].rearrange("p (two m) -> p two m", two=2),
            )
            nc.vector.memset(ebias, EXP_BIAS)

            # ---- phase A: x load (once, 16 tiles) THEN weights; stats on
            # the fly. DMA issues go on the sync+tensor queues (both idle
            # here) so GpSimd is free to be a third stats worker; the two
            # 8MB passes (sum, sum-of-squares) split ACT/DVE/GpSimd so the
            # head tracks the DMA arrival instead of one engine's serial
            # throughput. ----
            XT = 1024
            for cc in range(NCC):
                for jt in range(4):
                    eng = nc.sync if (cc * 4 + jt) % 2 == 0 else nc.gpsimd
                    eng.dma_start(
                        out=x_sb[:, cc, jt * XT : (jt + 1) * XT],
                        in_=xb.ap()[cc * P : (cc + 1) * P, jt * XT : (jt + 1) * XT],
                    )
            nc.sync.dma_start(out=gnw_t, in_=gnw.ap().rearrange("(a p) -> p a", p=P))
            nc.sync.dma_start(out=gnb_t, in_=gnb.ap().rearrange("(a p) -> p a", p=P))
            nc.gpsimd.dma_start(out=memb, in_=membd.ap())
            nc.gpsimd.dma_start(out=bcm, in_=bcd.ap())
            nc.sync.dma_start(out=wq8, in_=wq8d.ap())
            nc.gpsimd.dma_start(out=wk8, in_=wk8d.ap())
            nc.sync.dma_start(out=wv8, in_=wv8d.ap())
            nc.gpsimd.dma_start(out=wo8, in_=wo8d.ap())

            with (
                tc.tile_pool(name="statq", bufs=3) as sq_pool,
                tc.tile_pool(name="statsm", bufs=1) as sm,
                tc.tile_pool(name="statps", bufs=1, space="PSUM") as sps,
            ):
                gps = sps.tile([NG, 2], F32, tag="gstat")
                for cc in range(NCC):
                    s1t = sm.tile([P, 4], F32, tag=f"s1{cc}", name=f"s1{cc}")
                    s2t = sm.tile([P, 4], F32, tag=f"s2{cc}", name=f"s2{cc}")
                    for jt in range(4):
                        xsl = x_sb[:, cc, jt * XT : (jt + 1) * XT]
                        nc.vector.reduce_sum(
                            out=s1t[:, jt : jt + 1], in_=xsl, axis=mybir.AxisListType.X
                        )
                        sqw = sq_pool.tile([P, XT], F32, tag="sqw", name="sqw")
                        nc.scalar.activation(
                            out=sqw,
                            in_=xsl,
                            func=AF.Square,
                            accum_out=s2t[:, jt : jt + 1],
                        )
                    mm2 = sm.tile([P, 2], F32, tag=f"m2{cc}", name=f"m2{cc}")
                    m1r = sm.tile([P, 1], F32, tag=f"m1r{cc}", name=f"m1r{cc}")
                    nc.vector.reduce_sum(out=m1r, in_=s1t, axis=mybir.AxisListType.X)
                    nc.vector.tensor_scalar_mul(mm2[:, 0:1], m1r, 1.0 / HW)
                    m2r = sm.tile([P, 1], F32, tag=f"m2r{cc}", name=f"m2r{cc}")
                    nc.vector.reduce_sum(out=m2r, in_=s2t, axis=mybir.AxisListType.X)
                    nc.vector.tensor_scalar_mul(mm2[:, 1:2], m2r, 1.0 / HW)
                    nc.tensor.matmul(
                        gps,
                        memb[:, cc, :],
                        mm2,
                        start=(cc == 0),
                        stop=(cc == NCC - 1),
                    )
                # group stats -> per-channel scale/shift
                gs = sm.tile([NG, 2], F32, tag="gs")
                nc.scalar.mul(gs, gps, 1.0 / GS)
                sqg = sm.tile([NG, 1], F32, tag="sqg")
                nc.vector.tensor_mul(sqg, gs[:, 0:1], gs[:, 0:1])
                varg = sm.tile([NG, 1], F32, tag="varg")
                nc.vector.tensor_sub(varg, gs[:, 1:2], sqg)
                g2 = sm.tile([NG, 2], F32, tag="g2")
                nc.vector.tensor_copy(g2[:, 0:1], gs[:, 0:1])
                sbeps = sm.tile([NG, 1], F32, tag="eps")
                nc.vector.memset(sbeps, EPS)
                nc.scalar.activation(
                    out=g2[:, 1:2], in_=varg, func=AF.Sqrt, bias=sbeps
                )
                nc.vector.reciprocal(out=g2[:, 1:2], in_=g2[:, 1:2])
                for cc in range(NCC):
                    chp = sps.tile([P, 2], F32, tag="chs", name="chs")
                    nc.tensor.matmul(chp, bcm[:, cc, :], g2, start=True, stop=True)
                    nc.vector.tensor_mul(
                        gscale[:, cc : cc + 1], chp[:, 1:2], gnw_t[:, cc : cc + 1]
                    )
                    tmpm = sm.tile([P, 1], F32, tag="tm", name="tm")
                    nc.vector.tensor_mul(tmpm, chp[:, 0:1], gscale[:, cc : cc + 1])
                    nc.vector.tensor_sub(
                        gshift[:, cc : cc + 1], gnb_t[:, cc : cc + 1], tmpm
                    )

            # ---- phase C: projections (k, vT, q), all fp8 DoubleRow ----
            with (
                tc.tile_pool(name="projx", bufs=2) as px,
                tc.tile_pool(name="projps", bufs=3, space="PSUM") as pps,
            ):
                for jt in range(NJT):
                    jsl = slice(jt * JT, (jt + 1) * JT)
                    xn8 = px.tile([P, NCC, JT], F8, tag="xn", name="xn")
                    for cc in range(NCC):
                        # SBUF->SBUF, so GpSimd can own it (PSUM is off-limits
                        # to Pool); frees DVE/ACT for the PSUM->fp8 casts
                        nc.gpsimd.tensor_scalar(
                            out=xn8[:, cc, :],
                            in0=x_sb[:, cc, jsl],
                            scalar1=gscale[:, cc : cc + 1],
                            scalar2=gshift[:, cc : cc + 1],
                            op0=ALU.mult,
                            op1=ALU.add,
                        )
                    # k tiles (feature-major)
                    for oc in range(NCC):
                        kps = pps.tile([P, JT], F32, tag="kq", name="kps")
                        for c2 in range(2):
                            nc.tensor.matmul(
                                kps,
                                wk8[:, 2 * c2 : 2 * c2 + 2, oc * P : (oc + 1) * P],
                                xn8[:, 2 * c2 : 2 * c2 + 2, :],
                                start=(c2 == 0),
                                stop=(c2 == 1),
                                perf_mode=DR,
                            )
                        nc.scalar.mul(k8[:, oc, jsl], kps, 1.0 / WS)
                    # vT tiles (token-major)
                    for js in range(4):
                        vps = pps.tile([P, C], F32, tag="v", name="vps")
                        for c2 in range(2):
                            nc.tensor.matmul(
                                vps,
                                xn8[:, 2 * c2 : 2 * c2 + 2, js * P : (js + 1) * P],
                                wv8[:, 2 * c2 : 2 * c2 + 2, :],
                                start=(c2 == 0),
                                stop=(c2 == 1),
                                perf_mode=DR,
                            )
                        nc.vector.tensor_scalar_mul(
                            vT8[:, jt * 4 + js, :], vps, 1.0 / WS
                        )
                    # q tiles (first half only = our queries)
                    if jt < NJT // 2:
                        for oc in range(NCC):
                            qps = pps.tile([P, JT], F32, tag="kq", name="qps")
                            for c2 in range(2):
                                nc.tensor.matmul(
                                    qps,
                                    wq8[:, 2 * c2 : 2 * c2 + 2, oc * P : (oc + 1) * P],
                                    xn8[:, 2 * c2 : 2 * c2 + 2, :],
                                    start=(c2 == 0),
                                    stop=(c2 == 1),
                                    perf_mode=DR,
                                )
                            if oc % 2 == 0:
                                nc.scalar.mul(q8[:, oc, jsl], qps, 1.0 / WS)
                            else:
                                nc.vector.tensor_scalar_mul(
                                    q8[:, oc, jsl], qps, 1.0 / WS
                                )

            # ---- phase D: attention + out projection + residual ----
            with (
                tc.tile_pool(name="attne", bufs=3) as ae,
                tc.tile_pool(name="attnsb", bufs=2) as asb,
                tc.tile_pool(name="attnps", bufs=3, space="PSUM") as aps,
                tc.tile_pool(name="pvps", bufs=1, space="PSUM") as pvp_pool,
                tc.tile_pool(name="dnps", bufs=1, space="PSUM") as dnp_pool,
            ):
                pending_norm = None
                pending_tail = None
                for ig in range(NIG):
                    isl = slice(ig * IGW, (ig + 1) * IGW)
                    pvp = [
                        pvp_pool.tile([P, IGW], F32, tag=f"pv{cc}", name=f"pv{cc}")
                        for cc in range(NCC)
                    ]
                    dnp = dnp_pool.tile([P, IGW], F32, tag="dn")
                    for jc2 in range(NJC // 2):
                        expair = ae.tile([P, 2, IGW], F8, tag="ex", name="expair")
                        for hf in range(2):
                            jc = 2 * jc2 + hf
                            ap_t = aps.tile([P, IGW], F32, tag="attn", name="ap_t")
                            for c2 in range(2):
                                nc.tensor.matmul(
                                    ap_t,
                                    k8[:, 2 * c2 : 2 * c2 + 2, jc * P : (jc + 1) * P],
                                    q8[:, 2 * c2 : 2 * c2 + 2, isl],
                                    start=(c2 == 0),
                                    stop=(c2 == 1),
                                    perf_mode=DR,
                                )
                            nc.scalar.activation(
                                out=expair[:, hf, :],
                                in_=ap_t,
                                func=AF.Exp,
                                scale=EXP_SCALE,
                                bias=ebias,
                            )
                        if jc2 == 0 and pending_norm is not None:
                            pending_norm()
                            pending_norm = None
                        if jc2 == 2 and pending_tail is not None:
                            pending_tail()
                            pending_tail = None
                        nc.tensor.matmul(
                            dnp,
                            ones8,
                            expair,
                            start=(jc2 == 0),
                            stop=(jc2 == NJC // 2 - 1),
                            perf_mode=DR,
                        )
                        for cc in range(NCC):
                            nc.tensor.matmul(
                                pvp[cc],
                                vT8[:, 2 * jc2 : 2 * jc2 + 2, cc * P : (cc + 1) * P],
                                expair,
                                start=(jc2 == 0),
                                stop=(jc2 == NJC // 2 - 1),
                                perf_mode=DR,
                            )
                    # ---- ig boundary: drain pv PSUM to bf16 immediately (no
                    # dependency on the slow DVE reciprocal), then normalize
                    # from the staged copy inside the next ig's stream. dnp is
                    # already broadcast across partitions (ones8 lhsT).
                    pvraw = asb.tile([P, NCC, IGW], BF16, tag="pvraw", name="pvraw")
                    for cc in range(NCC):
                        nc.vector.tensor_copy(out=pvraw[:, cc, :], in_=pvp[cc])
                    pvn8 = asb.tile([P, NCC, IGW], F8, tag="pvn", name="pvn8")

                    def make_norm(dnp=dnp, pvraw=pvraw, pvn8=pvn8):
                        def norm():
                            recipb = asb.tile(
                                [P, IGW], F32, tag="recip", name="recipb"
                            )
                            nc.vector.reciprocal(out=recipb, in_=dnp)
                            for cc in range(NCC):
                                nc.vector.tensor_mul(
                                    pvn8[:, cc, :], pvraw[:, cc, :], recipb
                                )
                        return norm

                    def make_tail(isl=isl, pvn8=pvn8):
                        def tail():
                            for oc in range(NCC):
                                oop = aps.tile([P, IGW], F32, tag="attn", name="oop")
                                for c2 in range(2):
                                    nc.tensor.matmul(
                                        oop,
                                        wo8[
                                            :,
                                            2 * c2 : 2 * c2 + 2,
                                            oc * P : (oc + 1) * P,
                                        ],
                                        pvn8[:, 2 * c2 : 2 * c2 + 2, :],
                                        start=(c2 == 0),
                                        stop=(c2 == 1),
                                        perf_mode=DR,
                                    )
                                fo = asb.tile([P, IGW], F32, tag="fo", name="fo")
                                nc.vector.tensor_add(fo, oop, x_sb[:, oc, isl])
                                nc.gpsimd.dma_start(
                                    out=outd.ap()[oc * P : (oc + 1) * P, isl],
                                    in_=fo,
                                )
                        return tail

                    pending_norm = make_norm()
                    pending_tail = make_tail()
                pending_norm()
                pending_tail()

    return nc


_NC_CACHE = {}


def _get_module():
    if "nc" not in _NC_CACHE:
        nc = build()
        _split_drain_waits(nc)  # only needed for walrus codegen, not CoreSim
        _NC_CACHE["nc"] = nc
    return _NC_CACHE["nc"]


def _memb_np():
    m = np.zeros((P, NCC, NG), np.float32)
    for p in range(P):
        for cc in range(NCC):
            m[p, cc, cc * 8 + p // GS] = 1.0
    return m


def _bc_np():
    b = np.zeros((NG, NCC, P), np.float32)
    for cc in range(NCC):
        for p in range(P):
            b[cc * 8 + p // GS, cc, p] = 1.0
    return b


def _w8(w, scale):
    """w [C_out, C_in] f32 -> [P, NCC, C_out] fp8 tile: w8[p, cc, o] =
    (scale * w)[o, cc*128+p]."""
    wT = (np.asarray(w, np.float32) * scale).T  # [C_in, C_out]
    return np.ascontiguousarray(
        wT.reshape(NCC, P, C).transpose(1, 0, 2).astype(E4NP)
    )


def make_in_maps(inputs):
    x = np.asarray(inputs["x"], np.float32).reshape(B, C, HW)
    shared = {
        "wq8d": _w8(inputs["wq"], WS),
        "wk8d": _w8(inputs["wk"], WS),
        "wv8d": _w8(inputs["wv"], WS),
        "wo8d": _w8(inputs["wo"], 1.0),
        "gnw": np.ascontiguousarray(np.asarray(inputs["gn_w"], np.float32)),
        "gnb": np.ascontiguousarray(np.asarray(inputs["gn_b"], np.float32)),
        "membd": _memb_np(),
        "bcd": _bc_np(),
    }
    in_maps = []
    for core in range(8):
        b, h = core // 2, core % 2
        xbm = x[b]
        if h == 1:
            xbm = np.concatenate([xbm[:, HALF:], xbm[:, :HALF]], axis=1)
        in_maps.append({"xb": np.ascontiguousarray(xbm), **shared})
    return in_maps


def assemble(results):
    out = np.empty((B, C, HW), np.float32)
    for core in range(8):
        b, h = core // 2, core % 2
        out[b][:, h * HALF : (h + 1) * HALF] = results[core]["out"]
    return out.reshape(B, C, H, W)


def run_spmd(inputs, trace=False):
    nc = _get_module()
    res = run_bass_kernel_spmd(
        nc, make_in_maps(inputs), core_ids=list(range(8)), trace=trace
    )
    return assemble(res.results), res


def kernel(**inputs) -> np.ndarray:
    out, _ = run_spmd(inputs)
    return out
